# revision 52
# baseline (speedup 1.0000x reference)
"""DiffAttention (differential multi-head attention) Bass kernel for 8 TRN2 NeuronCores.

Sharding: the 16 differential heads are split across the 8 cores (2 per core).
wq/wk/wv are column-parallel (each core computes its heads' q/k/v projections
from the full, replicated activations), out_proj is row-parallel: each core
produces a full-shape partial output and the host sums the 8 partials.

Layout notes:
  - x is passed transposed (xT [E, T]) so q/k projections come out as
    qT/kT [head_dim, T] (needed as matmul operands for scores) without any
    on-chip transposes.
  - wq/wk rows are permuted host-side so each 64-dim head comes out
    de-interleaved ([32 real | 32 imag] RoPE halves). Scores are invariant to
    a common q/k head-dim permutation.
  - Softmax is unnormalized on-chip: e = exp(s), attention uses e directly and
    each head's value matrix carries an extra ones-column so the attn matmul
    produces both e@v and rowsum(e); normalization (and the diff-attn lambda
    combine + RMSNorm) happens on the small [t,128] attn tiles.
  - rsqrt for RMSNorm is computed as exp(-0.5*ln(x)) so the ScalarE only ever
    needs the natural_log_exp_and_others activation table (no table thrash
    with the softmax exp's).
  - fp32 matmuls run as float32r (full-rate for moving dim >= 256); all tiles
    feeding fp32r matmuls are declared float32r so their producers emit
    rounded outputs (BIR verifier requirement).
"""

import numpy as np

import concourse.bacc as bacc
import concourse.mybir as mybir
from concourse.tile import TileContext
from concourse.masks import make_identity
from concourse.bass_utils import run_bass_kernel_spmd

# Force every ScalarE activation onto the one table set that contains all the
# functions this kernel uses (Exp, Ln, Copy): natural_log_exp_and_others.
# The default chooser takes the first covering set per function, which
# alternates exp_and_others / natural_log and costs ~2.7us per switch.
_orig_get_tables = bacc.get_activation_tables


def _single_set_tables(arch):
    tabs = _orig_get_tables(arch)
    keep = "natural_log_exp_and_others"
    if keep in tabs:
        tabs = {k: (v if k == keep else set()) for k, v in tabs.items()}
    return tabs


bacc.get_activation_tables = _single_set_tables

E = 2048            # embed dim
T = 2048            # sequence length
HALF = 64           # q/k head dim
NH2 = 32            # q/k heads
H = 16              # differential heads
N_CORES = 8
HPC = H // N_CORES          # diff heads per core (2)
QPC = 2 * HPC               # q/k heads per core (4)
DPC = QPC * HALF            # q/k feature rows per core (256)
FPC = HPC * 2 * HALF        # v/attn feature cols per core (256)
DEPTH = 12
LAMBDA_INIT = 0.8 - 0.6 * float(np.exp(-0.3 * DEPTH))
SQRT_HD = float((2 * HALF) ** 0.5)   # scores are multiplied by sqrt(128)
EPS = 1e-5

F32 = mybir.dt.float32
F32R = mybir.dt.float32r
BF16 = mybir.dt.bfloat16
AF = mybir.ActivationFunctionType

TSUP = 512          # wide tile (moving free dim of most matmuls)
NT = T // TSUP      # 4
NE = E // 128       # 16 contraction chunks over embed dim
NS = T // 128       # 16 s (key position) chunks
TW = 1024           # scores/exp super-tile width (2 PSUM banks)
NTW = T // TW       # 2
VW = 2 * HALF + 2   # 130: v columns per head + ones column + pad (8B psum align)


def build_nc():
    nc = bacc.Bacc("TRN2", target_bir_lowering=False, debug=False)

    xT = nc.dram_tensor("xT", [E, T], F32R, kind="ExternalInput").ap()
    wqT = nc.dram_tensor("wqT", [E, DPC], F32R, kind="ExternalInput").ap()
    wkT = nc.dram_tensor("wkT", [E, DPC], F32R, kind="ExternalInput").ap()
    wvT = nc.dram_tensor("wvT", [E, FPC], F32R, kind="ExternalInput").ap()
    woutT = nc.dram_tensor("woutT", [FPC, E], F32R, kind="ExternalInput").ap()
    cosF = nc.dram_tensor("cosF", [128, T], F32, kind="ExternalInput").ap()
    sinS = nc.dram_tensor("sinS", [128, T], F32, kind="ExternalInput").ap()
    sublnc = nc.dram_tensor("sublnc", [FPC, 1], F32, kind="ExternalInput").ap()
    lq1 = nc.dram_tensor("lq1", [1, HALF], F32, kind="ExternalInput").ap()
    lk1 = nc.dram_tensor("lk1", [1, HALF], F32, kind="ExternalInput").ap()
    lq2 = nc.dram_tensor("lq2", [1, HALF], F32, kind="ExternalInput").ap()
    lk2 = nc.dram_tensor("lk2", [1, HALF], F32, kind="ExternalInput").ap()
    outT = nc.dram_tensor("outT", [E, T], F32, kind="ExternalOutput").ap()

    with TileContext(nc) as tc:
        with (
            tc.tile_pool(name="consts", bufs=1) as consts,
            tc.tile_pool(name="persist", bufs=1) as persist,
        ):
            # ---- constants ----
            ident = consts.tile([128, 128], F32, tag="ident")
            make_identity(nc, ident)

            # lambda scalar: exp(lq1.lk1) - exp(lq2.lk2) + LAMBDA_INIT
            lam_parts = []
            for nm, (qa, ka) in (("l1", (lq1, lk1)), ("l2", (lq2, lk2))):
                a = consts.tile([1, HALF], F32, tag=f"{nm}a")
                nc.sync.dma_start(out=a, in_=qa)
                b = consts.tile([1, HALF], F32, tag=f"{nm}b")
                nc.sync.dma_start(out=b, in_=ka)
                nc.vector.tensor_mul(out=a, in0=a, in1=b)
                s = consts.tile([1, 1], F32, tag=f"{nm}s")
                nc.vector.reduce_sum(out=s, in_=a, axis=mybir.AxisListType.X)
                nc.scalar.activation(out=s, in_=s, func=AF.Exp)
                lam_parts.append(s)
            lamv = consts.tile([1, 1], F32, tag="lamv")
            nc.vector.tensor_sub(out=lamv, in0=lam_parts[0], in1=lam_parts[1])
            nc.vector.tensor_scalar(
                out=lamv, in0=lamv, scalar1=float(LAMBDA_INIT), scalar2=None,
                op0=mybir.AluOpType.add,
            )
            ones_row = consts.tile([1, 128], F32, tag="ones_row")
            nc.vector.memset(ones_row, 1.0)
            eps_t = consts.tile([128, 1], F32, tag="eps_t")
            nc.vector.memset(eps_t, float(EPS))
            lam_bc = consts.tile([128, 1], F32, tag="lam_bc")
            with tc.tile_pool(name="cps", bufs=1, space="PSUM") as cps:
                lam_ps = cps.tile([128, 1], F32, tag="lam_ps")
                nc.tensor.matmul(lam_ps, lhsT=ones_row, rhs=lamv, start=True, stop=True)
                nc.vector.tensor_copy(out=lam_bc, in_=lam_ps)

            # ---- persistent activations ----
            qTr = [persist.tile([128, T], F32R, tag=f"qTr{i}", name=f"qTr{i}") for i in range(2)]
            kTr = [persist.tile([128, T], F32R, tag=f"kTr{i}", name=f"kTr{i}") for i in range(2)]
            v_ext = [persist.tile([128, HPC * VW], BF16, tag=f"vext{i}", name=f"vext{i}") for i in range(NS)]
            attnT = [persist.tile([128, T], F32R, tag=f"attnT{h}", name=f"attnT{h}") for h in range(HPC)]

            # ================= phase 1: q/k/v projections + RoPE =================
            with (
                tc.tile_pool(name="wpool", bufs=1) as wpool,
                tc.tile_pool(name="p1c", bufs=1) as p1c,
                tc.tile_pool(name="p1x", bufs=12) as p1x,
                tc.tile_pool(name="p1tmp", bufs=2) as p1tmp,
                tc.tile_pool(name="p1ps", bufs=1, space="PSUM") as p1ps,
            ):
                cos_t = p1c.tile([128, T], F32, tag="cos")
                sin_t = p1c.tile([128, T], F32, tag="sin")

                # whole weight matrices as single tiles, chunk-major in the
                # free dim: cols [i*W : (i+1)*W] = contraction chunk i.
                # Loaded in 4-chunk pieces so the first matmuls start early.
                wbig = {}
                wsrc = {"wq": wqT, "wk": wkT, "wv": wvT}
                for nm, w in (("wq", DPC), ("wk", DPC), ("wv", FPC)):
                    wbig[nm] = wpool.tile([128, NE * w], F32R, tag=nm, name=nm)

                def emit_w_piece(piece):
                    isl = slice(piece * 4, (piece + 1) * 4)
                    for nm in ("wq", "wk", "wv"):
                        nc.sync.dma_start(
                            out=wbig[nm].rearrange("p (i c) -> p i c", i=NE)[:, isl],
                            in_=wsrc[nm].rearrange("(i p) c -> p i c", p=128)[:, isl])

                emit_w_piece(0)

                for j in range(NT):
                    js = slice(j * TSUP, (j + 1) * TSUP)
                    Pq = [p1ps.tile([128, TSUP], F32, tag=f"pq{d}", name=f"pq{d}_{j}") for d in range(2)]
                    Pk = [p1ps.tile([128, TSUP], F32, tag=f"pk{d}", name=f"pk{d}_{j}") for d in range(2)]
                    Pv = [p1ps.tile([128, FPC], F32, tag=f"pv{sb}", name=f"pv{sb}_{j}") for sb in range(4)]
                    # x chunk-pair tiles: cols [c*TSUP:(c+1)*TSUP] = e-chunk 2*ip+c
                    xts = []
                    for ip in range(NE // 2):
                        xt = p1x.tile([128, 2 * TSUP], F32R, tag="x", name=f"x{j}_{ip}")
                        nc.sync.dma_start(
                            out=xt.rearrange("p (c t) -> p c t", c=2),
                            in_=xT[2 * ip * 128:(2 * ip + 2) * 128, js].rearrange(
                                "(c p) t -> p c t", p=128))
                        xts.append(xt)
                    if j == 0:
                        # queued after the first weight piece + x tiles so the
                        # first projection matmuls start as early as possible
                        for piece in range(1, 4):
                            emit_w_piece(piece)
                        nc.sync.dma_start(out=cos_t, in_=cosF)
                        nc.sync.dma_start(out=sin_t, in_=sinS)
                    for i in range(NE):
                        xsl = xts[i // 2][:, (i % 2) * TSUP:(i % 2 + 1) * TSUP]
                        for d in range(2):
                            wq_sl = wbig["wq"][:, i * DPC + d * 128:i * DPC + (d + 1) * 128]
                            wk_sl = wbig["wk"][:, i * DPC + d * 128:i * DPC + (d + 1) * 128]
                            nc.tensor.matmul(Pq[d], lhsT=wq_sl, rhs=xsl,
                                             start=(i == 0), stop=(i == NE - 1))
                            nc.tensor.matmul(Pk[d], lhsT=wk_sl, rhs=xsl,
                                             start=(i == 0), stop=(i == NE - 1))
                        for sb in range(4):
                            nc.tensor.matmul(Pv[sb], lhsT=xsl[:, sb * 128:(sb + 1) * 128],
                                             rhs=wbig["wv"][:, i * FPC:(i + 1) * FPC],
                                             start=(i == 0), stop=(i == NE - 1))
                    # --- RoPE: out = P*cos + swap32(P)*signed_sin ---
                    for src, dst in ((Pk[0], kTr[0]), (Pk[1], kTr[1]),
                                     (Pq[0], qTr[0]), (Pq[1], qTr[1])):
                        ps = p1tmp.tile([128, TSUP], F32, tag="ps", name=f"ps{j}")
                        nc.scalar.activation(out=ps, in_=src, func=AF.Copy)
                        swp = p1tmp.tile([128, TSUP], F32, tag="swp", name=f"swp{j}")
                        for gsel in range(4):
                            o = gsel * 32
                            so = o ^ 32
                            nc.gpsimd.tensor_copy(out=swp[o:o + 32, :], in_=ps[so:so + 32, :])
                        t1 = p1tmp.tile([128, TSUP], F32, tag="t1", name=f"t1_{j}")
                        nc.vector.tensor_mul(out=t1, in0=ps, in1=cos_t[:, js])
                        t2 = p1tmp.tile([128, TSUP], F32, tag="t2", name=f"t2_{j}")
                        nc.vector.tensor_mul(out=t2, in0=swp, in1=sin_t[:, js])
                        nc.vector.tensor_add(out=dst[:, js], in0=t1, in1=t2)
                    # --- v psum drain into bf16 v_ext (+ ones/pad columns) ---
                    for sb in range(4):
                        vt = v_ext[4 * j + sb]
                        for h in range(HPC):
                            nc.scalar.activation(out=vt[:, h * VW:h * VW + 128],
                                                 in_=Pv[sb][:, h * 128:(h + 1) * 128],
                                                 func=AF.Copy)
                            # col 128: ones (rowsum trick); col 129: pad
                            nc.gpsimd.memset(vt[:, h * VW + 128:h * VW + 129], 1.0)
                            nc.gpsimd.memset(vt[:, h * VW + 129:h * VW + 130], 0.0)

            # ============ phase 2/3: scores, softmax, attn, out projection ============
            with (
                tc.tile_pool(name="epool", bufs=50) as epool,
                tc.tile_pool(name="epi", bufs=4) as epi,
                tc.tile_pool(name="p2ps", bufs=2, space="PSUM") as p2ps,
                tc.tile_pool(name="p2pa", bufs=2, space="PSUM") as p2pa,
                tc.tile_pool(name="p2pt", bufs=1, space="PSUM") as p2pt,
                tc.tile_pool(name="p2po", bufs=1, space="PSUM") as p2po,
            ):
                # wout (transposed, per-core slice); subln is applied on the
                # transposed attention tiles (per-partition scalar there).
                wo = []
                sub_t = []
                for h in range(HPC):
                    wt = consts.tile([128, E], F32R, tag=f"wo{h}", name=f"wo{h}")
                    nc.sync.dma_start(out=wt, in_=woutT[h * 128:(h + 1) * 128, :])
                    wo.append(wt)
                    st = consts.tile([128, 1], F32, tag=f"sub{h}", name=f"sub{h}")
                    nc.sync.dma_start(out=st, in_=sublnc[h * 128:(h + 1) * 128, :])
                    sub_t.append(st)

                def emit_outproj(jj):
                    js = slice(jj * TSUP, (jj + 1) * TSUP)
                    for eb in range(NE):
                        O = p2po.tile([128, TSUP], F32, tag="out", name=f"o{jj}_{eb}")
                        for h in range(HPC):
                            nc.tensor.matmul(O, lhsT=wo[h][:, eb * 128:(eb + 1) * 128],
                                             rhs=attnT[h][:, js],
                                             start=(h == 0), stop=(h == HPC - 1))
                        Ob = epi.tile([128, TSUP], F32, tag="ob", name=f"ob{jj}_{eb}")
                        # the last block lands after the exp streams are done:
                        # ScalarE is idle there, VectorE is not
                        if jj == 2 * NTW - 1:
                            nc.scalar.activation(out=Ob, in_=O, func=AF.Copy)
                        else:
                            nc.vector.tensor_copy(out=Ob, in_=O)
                        nc.sync.dma_start(out=outT[eb * 128:(eb + 1) * 128, js], in_=Ob)

                def emit_attn_unit(j2, h, et, tb):
                    # both diff-attn component heads accumulate into one
                    # psum bank: [e0@{v|1} | e1@{v|1}]
                    A = p2pa.tile([128, 2 * VW], F32, tag="attn", name=f"a{j2}_{h}_{tb}")
                    for m in range(2):
                        for i in range(NS):
                            nc.tensor.matmul(
                                A[:, m * VW:(m + 1) * VW],
                                lhsT=et[(m, i)][:, tb * 128:(tb + 1) * 128],
                                rhs=v_ext[i][:, h * VW:(h + 1) * VW],
                                start=(i == 0), stop=(i == NS - 1))
                    # epilogue: normalize, diff, RMSNorm
                    rho0 = epi.tile([128, 1], F32, tag="rho0", name=f"r0_{j2}{h}{tb}")
                    nc.vector.reciprocal(out=rho0, in_=A[:, 128:129])
                    rho1 = epi.tile([128, 1], F32, tag="rho1", name=f"r1_{j2}{h}{tb}")
                    nc.vector.reciprocal(out=rho1, in_=A[:, VW + 128:VW + 129])
                    nc.vector.tensor_mul(out=rho1, in0=rho1, in1=lam_bc)
                    d0 = epi.tile([128, 128], F32, tag="d0", name=f"d0_{j2}{h}{tb}")
                    nc.vector.tensor_scalar_mul(out=d0, in0=A[:, 0:128], scalar1=rho0)
                    d1 = epi.tile([128, 128], F32, tag="d1", name=f"d1_{j2}{h}{tb}")
                    nc.vector.tensor_scalar_mul(out=d1, in0=A[:, VW:VW + 128], scalar1=rho1)
                    nc.vector.tensor_sub(out=d0, in0=d0, in1=d1)
                    sq = epi.tile([128, 128], F32, tag="sq", name=f"sq_{j2}{h}{tb}")
                    nc.vector.tensor_mul(out=sq, in0=d0, in1=d0)
                    ss = epi.tile([128, 1], F32, tag="ss", name=f"ss_{j2}{h}{tb}")
                    nc.vector.reduce_sum(out=ss, in_=sq, axis=mybir.AxisListType.X)
                    # rsqrt(mean+eps) = exp(-0.5*ln(sum/128 + eps))
                    nc.scalar.activation(out=ss, in_=ss, func=AF.Ln,
                                         bias=eps_t, scale=1.0 / 128)
                    nc.scalar.activation(out=ss, in_=ss, func=AF.Exp, scale=-0.5)
                    af = epi.tile([128, 128], F32, tag="af", name=f"af_{j2}{h}{tb}")
                    nc.vector.tensor_scalar_mul(out=af, in0=d0, scalar1=ss)
                    Tp = p2pt.tile([128, 128], F32, tag="tp", name=f"tp_{j2}{h}{tb}")
                    nc.tensor.transpose(Tp, af, ident)
                    tcol = (j2 * TW // 128 + tb) * 128
                    # transposed tile rows are attn features -> fold the
                    # per-feature subln weight in here (per-partition scalar)
                    nc.vector.tensor_scalar_mul(
                        out=attnT[h][:, tcol:tcol + 128], in0=Tp, scalar1=sub_t[h])
                    # out projection interleaves with the following streams
                    if h == HPC - 1 and tb == 3:
                        emit_outproj(2 * j2)
                    elif h == HPC - 1 and tb == 7:
                        emit_outproj(2 * j2 + 1)

                # Software pipeline: the attention units of head (j2,h) are
                # emitted interleaved into the front half of the NEXT head's
                # score/exp stream, so the PE's in-order stream alternates
                # ScalarE-paced score matmuls with dense attention matmuls.
                pending = None
                for j2 in range(NTW):
                    for h in range(HPC):
                        et = {}
                        idx = 0
                        for m in range(2):
                            g = 2 * h + m
                            gt, go = g // 2, 64 * (g % 2)
                            for i in range(NS):
                                S = p2ps.tile([128, TW], F32, tag="score", name=f"s{j2}_{h}_{m}_{i}")
                                for hf in range(2):
                                    ts = slice(j2 * TW + hf * TSUP, j2 * TW + (hf + 1) * TSUP)
                                    nc.tensor.matmul(
                                        S[:, hf * TSUP:(hf + 1) * TSUP],
                                        lhsT=kTr[gt][go:go + 64, i * 128:(i + 1) * 128],
                                        rhs=qTr[gt][go:go + 64, ts], start=True, stop=True)
                                e = epool.tile([128, TW], BF16, tag="e", name=f"e{j2}_{h}_{m}_{i}")
                                nc.scalar.activation(out=e, in_=S, func=AF.Exp, scale=SQRT_HD)
                                et[(m, i)] = e
                                if pending is not None and idx < 16 and idx % 2 == 1:
                                    pj2, ph, pet = pending
                                    emit_attn_unit(pj2, ph, pet, idx // 2)
                                idx += 1
                        pending = (j2, h, et)
                pj2, ph, pet = pending
                for tb in range(TW // 128):
                    emit_attn_unit(pj2, ph, pet, tb)

    nc.finalize()
    return nc


_NC_CACHE = []


def _get_nc():
    if not _NC_CACHE:
        _NC_CACHE.append(build_nc())
    return _NC_CACHE[0]


class _CachedSpmdRunner:
    """run_bass_kernel_spmd re-traces and re-jits the PJRT executable on every
    call; this runner builds the jitted shard_map once and reuses it."""

    def __init__(self, nc):
        import jax
        from jax.sharding import Mesh, PartitionSpec
        from concourse import bass2jax, mybir as _mb

        bass2jax.install_neuronx_cc_hook()
        self.nc = nc
        partition_name = nc.partition_id_tensor.name if nc.partition_id_tensor else None
        in_names, out_names, out_avals = [], [], []
        for alloc in nc.m.functions[0].allocations:
            if not isinstance(alloc, _mb.MemoryLocationSet):
                continue
            name = alloc.memorylocations[0].name
            if alloc.kind == "ExternalInput":
                if name != partition_name:
                    in_names.append(name)
            elif alloc.kind == "ExternalOutput":
                out_names.append(name)
                out_avals.append(jax.core.ShapedArray(
                    tuple(alloc.tensor_shape), _mb.dt.np(alloc.dtype)))
        self.in_names, self.out_names, self.out_avals = in_names, out_names, out_avals
        n_params = len(in_names)
        n_outs = len(out_names)
        all_names = in_names + out_names
        if partition_name is not None:
            all_names = all_names + [partition_name]

        def _body(*args):
            operands = list(args)
            if partition_name is not None:
                operands.append(bass2jax.partition_id_tensor())
            outs = bass2jax._bass_exec_p.bind(
                *operands,
                out_avals=tuple(out_avals),
                in_names=tuple(all_names),
                out_names=tuple(out_names),
                lowering_input_output_aliases=(),
                sim_require_finite=True,
                sim_require_nnan=True,
                nc=nc,
            )
            return tuple(outs)

        from jax.experimental.shard_map import shard_map
        devices = jax.devices()[:N_CORES]
        mesh = Mesh(np.asarray(devices), ("core",))
        in_specs = (PartitionSpec("core"),) * (n_params + n_outs)
        out_specs = (PartitionSpec("core"),) * n_outs
        self._fn = jax.jit(
            shard_map(_body, mesh=mesh, in_specs=in_specs, out_specs=out_specs,
                      check_rep=False),
            keep_unused=True,
        )
        self._jax = jax

    def concat_inputs(self, in_maps):
        args = [np.concatenate([np.asarray(m[n]) for m in in_maps], axis=0)
                for n in self.in_names]
        for av in self.out_avals:
            args.append(np.zeros((N_CORES * av.shape[0], *av.shape[1:]), av.dtype))
        return args

    def device_put(self, args):
        return [self._jax.device_put(a) for a in args]

    def run(self, args):
        outs = self._fn(*args)
        return [np.asarray(o) for o in outs]

    def __call__(self, in_maps):
        outs = self.run(self.concat_inputs(in_maps))
        per_core = []
        for c in range(N_CORES):
            d = {}
            for i, n in enumerate(self.out_names):
                d[n] = outs[i].reshape(N_CORES, *self.out_avals[i].shape)[c]
            per_core.append(d)
        return per_core


_RUNNER_CACHE = []


def _get_runner():
    if not _RUNNER_CACHE:
        _RUNNER_CACHE.append(_CachedSpmdRunner(_get_nc()))
    return _RUNNER_CACHE[0]


def _prep_inputs(x, wq, wk, wv, wout, lambda_q1, lambda_q2, lambda_k1, lambda_k2,
                 subln_weight):
    x = np.asarray(x, np.float32).reshape(T, E)
    xT = np.ascontiguousarray(x.T)

    inv = 1.0 / (10000.0 ** (np.arange(0, HALF, 2)[: HALF // 2].astype(np.float64) / HALF))
    ang = np.outer(np.arange(T), inv)          # [T, 32]
    cos32 = np.cos(ang).T.astype(np.float32)   # [32, T]
    sin32 = np.sin(ang).T.astype(np.float32)
    cosF = np.ascontiguousarray(np.tile(cos32, (4, 1)), np.float32)
    sinS = np.ascontiguousarray(
        np.concatenate([-sin32, sin32, -sin32, sin32], axis=0), np.float32)

    subc = np.ascontiguousarray(
        np.tile(np.asarray(subln_weight, np.float32), HPC)[:, None])

    evens = np.arange(0, HALF, 2)
    odds = np.arange(1, HALF, 2)
    deint = np.concatenate([evens, odds])

    lam_ins = dict(
        lq1=np.asarray(lambda_q1, np.float32).reshape(1, HALF),
        lk1=np.asarray(lambda_k1, np.float32).reshape(1, HALF),
        lq2=np.asarray(lambda_q2, np.float32).reshape(1, HALF),
        lk2=np.asarray(lambda_k2, np.float32).reshape(1, HALF),
    )

    wq = np.asarray(wq, np.float32)
    wk = np.asarray(wk, np.float32)
    wv = np.asarray(wv, np.float32)
    wout = np.asarray(wout, np.float32)

    in_maps = []
    for c in range(N_CORES):
        perm = np.concatenate([g * HALF + deint for g in range(QPC * c, QPC * (c + 1))])
        in_maps.append(dict(
            xT=xT,
            wqT=np.ascontiguousarray(wq[perm, :].T),
            wkT=np.ascontiguousarray(wk[perm, :].T),
            wvT=np.ascontiguousarray(wv[FPC * c:FPC * (c + 1), :].T),
            woutT=np.ascontiguousarray(wout[:, FPC * c:FPC * (c + 1)].T),
            cosF=cosF, sinS=sinS, sublnc=subc, **lam_ins,
        ))
    return in_maps


def kernel(**inputs):
    runner = _get_runner()
    in_maps = _prep_inputs(**inputs)
    results = runner(in_maps)
    acc = results[0]["outT"].copy()
    for c in range(1, N_CORES):
        acc += results[c]["outT"]
    return np.ascontiguousarray(acc.T).reshape(1, T, E).astype(np.float32)
